# revision 1
# baseline (speedup 1.0000x reference)
"""ComplexAttentionLayer Trainium2 kernel, v2 (8-core data-parallel).

Math (per token t, head h; E=64; the per-head feature dim is 1, so scores
are outer products over the E axis):
  abs2[l,s] = 0.5*(Gp[l]*Hm[s] + Gm[l]*Hp[s]),  Gp=(qr+qi)^2, Gm=(qr-qi)^2,
                                                Hp=(kr+ki)^2, Hm=(kr-ki)^2
  attn = softmax(sqrt(abs2), axis=s)   (unnormalized exp; scores >= 0 and
         bounded far below f32 exp overflow, so no max subtraction)
  out[l] = sum_s attn[l,s] * v[s]

v2 structure: the score outer products and the attn@v contraction both run
on the tensor engine.  Tokens are processed in pairs (p, p+64) of a
128-token tile; per (token-tile, head, 32-pair chunk):
  - abs2: lhsT = H-staging [K=128 (4 used rows), (blk,s)=128], rhs =
    G-staging rows 0-3 = (GpA, GmA, GpB, GmB) -> psum [(blk,s), l]
  - scalar engine: E = exp(exp(0.5*ln(0.5*S)))  (single ln/exp table set)
  - contraction: lhsT = E-pair [(blk,s), l], rhs = V-staging [(blk,s), 6]
    with columns (vrA, viA, 1A, vrB, viB, 1B) -> psum [l, (pair, 6)]
    landing s.t. partitions = h-major output index d' = h*64+l
  - normalize with reciprocal of the ones-column sums, write transposed
    V directly in the output projection's lhsT layout.
sqrt is computed as exp(0.5*ln(x)) so the scalar engine never switches
activation-table sets.
"""

import numpy as np

import concourse.bass as bass
import concourse.tile as tile
from concourse import bacc, mybir
from concourse.bass_utils import run_bass_kernel_spmd
from concourse.masks import make_identity

AF = mybir.ActivationFunctionType
ALU = mybir.AluOpType
F32 = mybir.dt.float32
BF16 = mybir.dt.bfloat16

B, L, D, H = 4, 1024, 512, 8
E = D // H  # 64
NCORES = 8
T = B * L // NCORES  # 512 tokens per core
PT = 128             # tokens per tile
NTT = T // PT        # 4 token tiles per core
KT = D // 128        # 4 k-tiles per weight
NPAIR = 64           # token pairs (p, p+64) per tile
CH = 32              # pairs per staging chunk
HC = 8               # pairs per abs2-psum sub chunk


def _patch_act_tables():
    """Keep Ln/Exp/Copy/Identity only in natural_log_exp_and_others so the
    table-load pass picks the one set covering all our activations (one
    ACT_TABLE_LOAD instead of ping-ponging between per-function sets)."""
    from concourse import hw_specs
    orig = hw_specs.get_activation_tables
    AFT = mybir.ActivationFunctionType
    ours = {AFT.Ln, AFT.Exp, AFT.Copy, AFT.Identity}

    def patched(module_arch):
        tabs = orig(module_arch)
        if "natural_log_exp_and_others" in tabs:
            for name, fns in tabs.items():
                if name != "natural_log_exp_and_others":
                    tabs[name] = fns - ours
        return tabs

    bacc.get_activation_tables = patched


def _build_module():
    nc = bacc.Bacc()

    xT = {}
    for nm in ("q_r", "q_i", "k_r", "k_i", "v_r", "v_i"):
        xT[nm] = nc.declare_dram_parameter(f"x_{nm}_T", [D, T], BF16, isOutput=False)
    w = {}
    for p in ("q", "k", "v", "o"):
        for c in ("r", "i", "in"):  # r = w_r.T, i = w_i.T, in = -w_i.T
            w[p, c] = nc.declare_dram_parameter(f"w_{p}_{c}", [D, D], BF16,
                                                isOutput=False)
    bias = {}
    for p in ("q", "k", "v", "o"):
        for c in ("r", "i"):  # r: br-bi, i: br+bi
            bias[p, c] = nc.declare_dram_parameter(f"b_{p}_{c}", [1, D], BF16,
                                                   isOutput=False)
    out_r = nc.declare_dram_parameter("out_r", [T, D], F32, isOutput=True)
    out_i = nc.declare_dram_parameter("out_i", [T, D], F32, isOutput=True)

    with tile.TileContext(nc) as tc:
        with (
            tc.tile_pool(name="const", bufs=1) as const_pool,
            tc.tile_pool(name="xin", bufs=2) as x_pool,
            tc.tile_pool(name="wgt", bufs=2) as w_pool,
            tc.tile_pool(name="score", bufs=2) as score_pool,
            tc.tile_pool(name="acc", bufs=2) as acc_pool,
            tc.tile_pool(name="evac", bufs=3) as evac_pool,
            tc.tile_pool(name="psum", bufs=2, space="PSUM") as psum_pool,
            tc.tile_pool(name="psum_a", bufs=2, space="PSUM") as psum_a_pool,
            tc.tile_pool(name="psum_c", bufs=2, space="PSUM") as psum_c_pool,
        ):
            ident = const_pool.tile([128, 128], F32, tag="ident")
            make_identity(nc, ident[:])
            ones_row = const_pool.tile([1, 128], BF16, tag="ones")
            nc.gpsimd.memset(ones_row[:], 1.0)

            bs = {}
            for key, dram in bias.items():
                t = const_pool.tile([1, D], BF16, name=f"b_{key[0]}_{key[1]}",
                                    tag=f"b_{key[0]}_{key[1]}")
                nc.sync.dma_start(t[:], dram[:])
                bs[key] = t

            def load_kxn(pool, dram, tag, n, eng=None):
                t = pool.tile([128, KT, n], BF16, name=tag, tag=tag)
                (eng or nc.sync).dma_start(
                    t[:], dram[:].rearrange("(k p) n -> p k n", p=128))
                return t

            # G/H/v for the whole core, h-major free layout: d' = h*64 + e
            ghv = {nm: const_pool.tile([PT, NTT, D],
                                       F32 if nm[0] == "v" else BF16,
                                       name=f"ghv_{nm}", tag=f"ghv_{nm}")
                   for nm in ("Gp", "Gm", "Hp", "Hm", "vr", "vi")}

            def hmaj(ap):
                # natural [p, (l h)] view -> h-major [p, l, h] target strides
                return ap.rearrange("p (h l) -> p l h", l=E)

            def nat(ap):
                return ap.rearrange("p (l h) -> p l h", h=H)

            def cproj(wset, xr_t, xi_t, tt):
                """complex linear on token tile tt -> (psum_yr, psum_yi)"""
                ts = bass.ts(tt, PT)
                yr = psum_pool.tile([PT, D], F32, tag="mm")
                yi = psum_pool.tile([PT, D], F32, tag="mm")
                for k in range(KT):
                    nc.tensor.matmul(yr[:], xr_t[:, k, ts], wset["r"][:, k, :],
                                     start=(k == 0), stop=False)
                for k in range(KT):
                    nc.tensor.matmul(yr[:], xi_t[:, k, ts], wset["in"][:, k, :],
                                     start=False, stop=False)
                nc.tensor.matmul(yr[:], ones_row[:], wset["br"][:],
                                 start=False, stop=True)
                for k in range(KT):
                    nc.tensor.matmul(yi[:], xi_t[:, k, ts], wset["r"][:, k, :],
                                     start=(k == 0), stop=False)
                for k in range(KT):
                    nc.tensor.matmul(yi[:], xr_t[:, k, ts], wset["i"][:, k, :],
                                     start=False, stop=False)
                nc.tensor.matmul(yi[:], ones_row[:], wset["bi"][:],
                                 start=False, stop=True)
                return yr, yi

            # ---- phase 1: q/k/v projections -> G/H/v (h-major) ----
            for p, (sum_nm, dif_nm) in (("q", ("Gp", "Gm")),
                                        ("k", ("Hp", "Hm")),
                                        ("v", ("vr", "vi"))):
                eng = nc.sync if p == "q" else nc.gpsimd
                xr_t = load_kxn(x_pool, xT[f"{p}_r"], "xr", T, eng)
                xi_t = load_kxn(x_pool, xT[f"{p}_i"], "xi", T, eng)
                wset = {c: load_kxn(w_pool, w[p, c], f"w{c}", D, eng)
                        for c in ("r", "i", "in")}  # noqa
                wset["br"] = bs[p, "r"]
                wset["bi"] = bs[p, "i"]
                for tt in range(NTT):
                    yr, yi = cproj(wset, xr_t, xi_t, tt)
                    if p == "v":
                        nc.scalar.copy(hmaj(ghv["vr"][:, tt, :]), nat(yr[:]))
                        nc.scalar.copy(hmaj(ghv["vi"][:, tt, :]), nat(yi[:]))
                    else:
                        yr_s = evac_pool.tile([PT, D], F32, tag="evac")
                        nc.vector.tensor_copy(yr_s[:], yr[:])
                        yi_s = evac_pool.tile([PT, D], F32, tag="evac")
                        nc.vector.tensor_copy(yi_s[:], yi[:])
                        tp = evac_pool.tile([PT, D], F32, tag="evac")
                        nc.vector.tensor_add(tp[:], yr_s[:], yi_s[:])
                        nc.vector.tensor_mul(hmaj(ghv[sum_nm][:, tt, :]),
                                             nat(tp[:]), nat(tp[:]))
                        tm = evac_pool.tile([PT, D], F32, tag="evac")
                        nc.vector.tensor_sub(tm[:], yr_s[:], yi_s[:])
                        nc.vector.tensor_mul(hmaj(ghv[dif_nm][:, tt, :]),
                                             nat(tm[:]), nat(tm[:]))

            # ---- phase 2: attention (PE outer products + PE contraction) --
            # output-projection operand, h-major: row d' = h*64+l ->
            # partition (h%2)*64+l, plane h//2; col = token
            VrT = const_pool.tile([128, KT, T], BF16, tag="VrT")
            ViT = const_pool.tile([128, KT, T], BF16, tag="ViT")

            # staging buffers (manual ping-pong)
            hst = [const_pool.tile([128, CH * 128], BF16, name=f"hst{i}",
                                   tag=f"hst{i}") for i in range(2)]
            gst = [const_pool.tile([128, CH * E], BF16, name=f"gst{i}",
                                   tag=f"gst{i}") for i in range(2)]
            vst = [const_pool.tile([128, CH * 6], BF16, name=f"vst{i}",
                                   tag=f"vst{i}") for i in range(2)]
            for i in range(2):
                nc.gpsimd.memset(hst[i][:], 0.0)
                nc.gpsimd.memset(gst[i][:], 0.0)
                nc.gpsimd.memset(vst[i][:], 0.0)
                on6 = vst[i][:].rearrange("p (j c) -> p j c", c=6)
                nc.gpsimd.memset(on6[0:64, :, 2], 1.0)
                nc.gpsimd.memset(on6[64:128, :, 5], 1.0)

            chunk_idx = 0
            for tt in range(NTT):
                for h in range(H):
                    hs = slice(h * E, (h + 1) * E)
                    base = 64 * (h % 2)
                    # V transpose for this (tt, h) via regular matmul
                    # against the identity (transpose-mode can't write at
                    # partition 64): out = v.T @ I.  cols 0:64 = vr,
                    # 64:128 = vi; rows 0:64 = A tokens, 64:128 = B tokens
                    vtr = psum_c_pool.tile([128, 128], F32, tag="ctr")
                    nc.tensor.matmul(vtr[0:64, 0:64],
                                     ghv["vr"][0:64, tt, hs],
                                     ident[0:64, 0:64])
                    nc.tensor.matmul(vtr[64:128, 0:64],
                                     ghv["vr"][64:128, tt, hs],
                                     ident[64:128, 64:128])
                    nc.tensor.matmul(vtr[0:64, 64:128],
                                     ghv["vi"][0:64, tt, hs],
                                     ident[0:64, 0:64])
                    nc.tensor.matmul(vtr[64:128, 64:128],
                                     ghv["vi"][64:128, tt, hs],
                                     ident[64:128, 64:128])

                    ctr = psum_c_pool.tile([128, NPAIR * 6], F32, tag="bigctr")
                    Et_full = score_pool.tile([128, NPAIR * E], BF16, tag="Et")

                    for c in range(2):  # staging chunks of CH=32 pairs
                        sl32 = slice(c * CH, (c + 1) * CH)
                        sl32b = slice(64 + c * CH, 64 + (c + 1) * CH)
                        hb = hst[chunk_idx % 2]
                        gb = gst[chunk_idx % 2]
                        vb = vst[chunk_idx % 2]
                        chunk_idx += 1
                        # G staging rows 0-3 = GpA, GmA, GpB, GmB
                        for r, (nm, ts_) in enumerate(
                                (("Gp", sl32), ("Gm", sl32),
                                 ("Gp", sl32b), ("Gm", sl32b))):
                            dv = gb[r:r + 1, 0:CH * E]
                            nc.sync.dma_start(dv, ghv[nm][ts_, tt, hs])
                        # H staging rows 0-3 = HmA, HpA (cols 0:64),
                        #                      HmB, HpB (cols 64:128)
                        for r, (nm, ts_, half) in enumerate(
                                (("Hm", sl32, 0), ("Hp", sl32, 0),
                                 ("Hm", sl32b, 1), ("Hp", sl32b, 1))):
                            dv = hb[r:r + 1, :].rearrange(
                                "p (j two s) -> p j two s",
                                two=2, s=E)[:, :, half, :]
                            nc.sync.dma_start(dv, ghv[nm][ts_, tt, hs])
                        # V staging: cols 6j + (0,1,2 | 3,4,5) =
                        #            (vrA, viA, 1) | (vrB, viB, 1)
                        v6 = vb[:].rearrange("p (j c) -> p j c", c=6)
                        slv = slice(c * CH, (c + 1) * CH)
                        slvi = slice(64 + c * CH, 64 + (c + 1) * CH)
                        nc.vector.tensor_copy(v6[0:64, :, 0], vtr[0:64, slv])
                        nc.vector.tensor_copy(v6[0:64, :, 1], vtr[0:64, slvi])
                        nc.vector.tensor_copy(v6[64:128, :, 3],
                                              vtr[64:128, slv])
                        nc.vector.tensor_copy(v6[64:128, :, 4],
                                              vtr[64:128, slvi])

                        sA = score_pool.tile([128, CH * E], F32, tag="sA")
                        for hc in range(CH // HC):  # abs2 sub chunks of HC
                            ab = psum_a_pool.tile([128, HC * E], F32,
                                                  tag="abs2")
                            for jl in range(HC):
                                j = hc * HC + jl
                                nc.tensor.matmul(
                                    ab[:, bass.ts(jl, E)],
                                    hb[:, bass.ts(j, 128)],
                                    gb[:, bass.ts(j, E)])
                            # PE result is 2*abs2; ln(0.5 * PE) = ln(abs2)
                            nc.scalar.activation(
                                sA[:, bass.ts(hc, HC * E)], ab[:],
                                AF.Ln, scale=0.5)
                        sB = score_pool.tile([128, CH * E], F32, tag="sB")
                        nc.scalar.activation(sB[:], sA[:], AF.Exp, scale=0.5)
                        nc.scalar.activation(
                            Et_full[:, bass.ts(c, CH * E)], sB[:], AF.Exp)

                        # contraction for this chunk
                        for jl in range(CH):
                            j = c * CH + jl
                            nc.tensor.matmul(
                                ctr[base:base + 64, bass.ts(j, 6)],
                                Et_full[:, bass.ts(j, E)],
                                vb[:, bass.ts(jl, 6)])

                    # normalize + write transposed V (bf16) for o-proj
                    c6 = ctr[base:base + 64, :].rearrange(
                        "p (j blk c) -> p j blk c", blk=2, c=3)
                    rcp = acc_pool.tile([64, 128], F32, tag="rcp")
                    nc.vector.reciprocal(rcp[:].rearrange(
                        "p (blk j) -> p j blk", blk=2), c6[:, :, :, 2])
                    # VrT view: partition base..base+64, plane h//2,
                    # cols (blk*64 + j) + tt*128
                    vr_dst = VrT[base:base + 64, h // 2,
                                 tt * PT:(tt + 1) * PT].rearrange(
                                     "p (blk j) -> p j blk", blk=2)
                    vi_dst = ViT[base:base + 64, h // 2,
                                 tt * PT:(tt + 1) * PT].rearrange(
                                     "p (blk j) -> p j blk", blk=2)
                    rcpv = rcp[:].rearrange("p (blk j) -> p j blk", blk=2)
                    nc.vector.tensor_mul(vr_dst, c6[:, :, :, 0], rcpv)
                    nc.vector.tensor_mul(vi_dst, c6[:, :, :, 1], rcpv)

            # ---- phase 3: output projection ----
            wo = {c: load_kxn(w_pool, w["o", c], f"w{c}", D, nc.gpsimd)
                  for c in ("r", "i", "in")}
            wo["br"] = bs["o", "r"]
            wo["bi"] = bs["o", "i"]
            for tt in range(NTT):
                ts = bass.ts(tt, PT)
                our = psum_pool.tile([PT, D], F32, tag="mm")
                oui = psum_pool.tile([PT, D], F32, tag="mm")
                for k in range(KT):
                    nc.tensor.matmul(our[:], VrT[:, k, ts], wo["r"][:, k, :],
                                     start=(k == 0), stop=False)
                for k in range(KT):
                    nc.tensor.matmul(our[:], ViT[:, k, ts], wo["in"][:, k, :],
                                     start=False, stop=False)
                nc.tensor.matmul(our[:], ones_row[:], wo["br"][:],
                                 start=False, stop=True)
                for k in range(KT):
                    nc.tensor.matmul(oui[:], ViT[:, k, ts], wo["r"][:, k, :],
                                     start=(k == 0), stop=False)
                for k in range(KT):
                    nc.tensor.matmul(oui[:], VrT[:, k, ts], wo["i"][:, k, :],
                                     start=False, stop=False)
                nc.tensor.matmul(oui[:], ones_row[:], wo["bi"][:],
                                 start=False, stop=True)

                sor = evac_pool.tile([PT, D], F32, tag="sor")
                soi = evac_pool.tile([PT, D], F32, tag="soi")
                nc.scalar.copy(sor[:], our[:])
                nc.scalar.copy(soi[:], oui[:])
                nc.gpsimd.dma_start(out_r[ts, :], sor[:])
                nc.gpsimd.dma_start(out_i[ts, :], soi[:])

    nc.compile()
    return nc


_NC_CACHE = None


def _get_module():
    global _NC_CACHE
    if _NC_CACHE is None:
        _patch_act_tables()
        _NC_CACHE = _build_module()
    return _NC_CACHE


def _prep_inputs(inputs):
    """host-side shard/layout prep -> list of 8 per-core input maps"""
    import ml_dtypes
    bf = ml_dtypes.bfloat16
    TT = B * L
    xs = {nm: np.ascontiguousarray(
        np.asarray(inputs[nm]).reshape(TT, D).T.astype(bf))
        for nm in ("q_r", "q_i", "k_r", "k_i", "v_r", "v_i")}
    # o-projection contracts over d in h-major order d' = h*64 + l:
    # permute the corresponding weight rows
    perm = np.empty(D, np.int64)
    for h in range(H):
        for l in range(E):
            perm[h * E + l] = l * H + h
    common = {}
    for p in ("q", "k", "v", "o"):
        wr = np.asarray(inputs[f"w{p}_r"]).astype(np.float32)
        wi = np.asarray(inputs[f"w{p}_i"]).astype(np.float32)
        br = np.asarray(inputs[f"b{p}_r"]).astype(np.float32)
        bi = np.asarray(inputs[f"b{p}_i"]).astype(np.float32)
        wrT = wr.T
        wiT = wi.T
        if p == "o":
            wrT = wrT[perm, :]
            wiT = wiT[perm, :]
        common[f"w_{p}_r"] = np.ascontiguousarray(wrT.astype(bf))
        common[f"w_{p}_i"] = np.ascontiguousarray(wiT.astype(bf))
        common[f"w_{p}_in"] = np.ascontiguousarray((-wiT).astype(bf))
        common[f"b_{p}_r"] = (br - bi).reshape(1, D).astype(bf)
        common[f"b_{p}_i"] = (br + bi).reshape(1, D).astype(bf)
    maps = []
    for c in range(NCORES):
        m = dict(common)
        sl = slice(c * T, (c + 1) * T)
        for nm, arr in xs.items():
            m[f"x_{nm}_T"] = np.ascontiguousarray(arr[:, sl])
        maps.append(m)
    return maps


def kernel(**inputs):
    nc = _get_module()
    maps = _prep_inputs(inputs)
    res = run_bass_kernel_spmd(nc, maps, core_ids=list(range(NCORES)))
    out_r = np.concatenate([res.results[c]["out_r"] for c in range(NCORES)],
                           axis=0).reshape(B, L, D)
    out_i = np.concatenate([res.results[c]["out_i"] for c in range(NCORES)],
                           axis=0).reshape(B, L, D)
    return out_r, out_i



# revision 18
# speedup vs baseline: 2.1817x; 2.1817x over previous
"""ComplexAttentionLayer Trainium2 kernel, v3 (8-core data-parallel).

Math (per token t, head h; E=64): with q = qr+i*qi, k = kr+i*ki per (h,l):
  m = qr^2+qi^2, p = 2*qr*qi, n = kr^2+ki^2, t2 = 2*kr*ki
  |s[l,s]|^2 = m[l]*n[s] - p[l]*t2[s] = (a_l b_s)^2 (1 - u_l v_s),
  a = sqrt(m), b = sqrt(n), u = p/m, v = t2/n  (|u|,|v| <= 1)
  attn = softmax_s exp(|s|), out = attn @ v.

Key trick: |s| = a_l b_s sqrt(1 - u_l v_s) is approximated by a degree-8
polynomial sqrt(1-z) ~= sum_k c_k z^k (max end-to-end rel err ~2.4e-3 in
fp16), making |s| itself a rank-9 PE matmul over per-token factor vectors
  g_k[l] = c_k a u^k (q side),  h_k[s] = b v^k (k side).
The scalar engine then does a SINGLE exp pass over the 16.8M scores
(vs ln/exp/exp in v2), and no per-element sqrt exists anywhere.

Layouts: q/k projections natural [token-part, feat]; v and o projections
transposed [feat-part, token] (weights permuted host-side to h-major
d' = h*64 + e). Per (token-tile, head-pair hp) the score matmul is
  out[(h2,s), l] = hst[K=18, (h2,s)].T @ gst[K=18, (j,l)]
with hst rows (h2,k) = h_k of head 2hp+h2 (zero off-block) staged by 3-dim
batched SBUF->SBUF DMAs from chain-multiply outputs, and the contraction
  ctr[l, (j,3)] += Et[(h2,s), j-slice].T @ vT3[(h2,s), j*3:(j+1)*3]
uses v in transposed layout with (vr, vi, 1) interleaved columns; the ones
column yields the softmax denominators.
"""

import numpy as np

import concourse.bass as bass
import concourse.tile as tile
from concourse import bacc, mybir
from concourse.bass_utils import run_bass_kernel_spmd

AF = mybir.ActivationFunctionType
ALU = mybir.AluOpType
F32 = mybir.dt.float32
F16 = mybir.dt.float16

B, L, D, H = 4, 1024, 512, 8
E = D // H           # 64
NCORES = 8
T = B * L // NCORES  # 512 tokens per core
PT = 128             # tokens per tile
NTT = T // PT        # 4 token tiles
KT = D // 128        # 4 k-tiles per weight
NHP = H // 2         # 4 head pairs
R = 9                # score rank (poly degree 8)
CH = 64              # tokens per staging chunk
NCH = PT // CH       # 2 chunks per token tile
RND = 16             # tokens per psum round

# degree-8 lstsq fit of sqrt(1-z) on [-1,1] (see module docstring)
COEF = [0.9990085414, -0.4886256302, -0.0785392818, -0.1987831962,
        -0.3640738545, 0.3670958221, 0.6909310557, -0.3454001713,
        -0.5032298768]
RATIO = [COEF[k + 1] / COEF[k] for k in range(R - 1)]


def _patch_act_tables():
    """Keep Ln/Exp/Copy/Identity only in natural_log_exp_and_others so the
    table-load pass picks the one set covering all our activations (this
    kernel only uses Ln and Exp, which genuinely share that table)."""
    from concourse import hw_specs
    orig = hw_specs.get_activation_tables
    ours = {AF.Ln, AF.Exp, AF.Copy, AF.Identity}

    def patched(module_arch):
        tabs = orig(module_arch)
        if "natural_log_exp_and_others" in tabs:
            for name, fns in tabs.items():
                if name != "natural_log_exp_and_others":
                    tabs[name] = fns - ours
        return tabs

    bacc.get_activation_tables = patched


def _build_module():
    nc = bacc.Bacc()

    xT = {}
    for nm in ("q_r", "q_i", "k_r", "k_i", "v_r", "v_i"):
        xT[nm] = nc.declare_dram_parameter(f"x_{nm}_T", [D, T], F16,
                                           isOutput=False)
    w = {}
    for p in ("q", "k"):
        # p = (wr+wi).T, m = (wr-wi).T, n = -(wr+wi).T  (sum/diff form so the
        # projection outputs s1 = yr+yi, s2 = yr-yi directly)
        for c in ("p", "m", "n"):
            w[p, c] = nc.declare_dram_parameter(f"w_{p}_{c}", [D, D], F16,
                                                isOutput=False)
    for p in ("v", "o"):
        for c in ("r", "i", "in"):  # r = w_r.T, i = w_i.T, in = -w_i.T
            w[p, c] = nc.declare_dram_parameter(f"w_{p}_{c}", [D, D], F16,
                                                isOutput=False)
    bias_all = nc.declare_dram_parameter("bias_all", [1, 8 * D], F16,
                                         isOutput=False)
    # DRAM bounce scratch for the partition->free staging transpose
    # (SBUF->SBUF DMAs cannot move data across partitions multi-dim)
    scr = {s: nc.declare_dram_parameter(f"scr_{s}", [NTT, 128, NHP * 2 * R * E],
                                        F16, isOutput=True)
           for s in ("q", "k")}
    outT_r = nc.declare_dram_parameter("outT_r", [D, T], F16, isOutput=True)
    outT_i = nc.declare_dram_parameter("outT_i", [D, T], F16, isOutput=True)

    with tile.TileContext(nc) as tc:
        with (
            tc.tile_pool(name="const", bufs=1) as const_pool,
            tc.tile_pool(name="xin", bufs=2) as x_pool,
            tc.tile_pool(name="wgt", bufs=2) as w_pool,
            tc.tile_pool(name="prep", bufs=2) as prep_pool,
            tc.tile_pool(name="gh", bufs=2) as gh_pool,
            tc.tile_pool(name="stage", bufs=2) as stage_pool,
            tc.tile_pool(name="et", bufs=3) as et_pool,
            tc.tile_pool(name="vout", bufs=1) as vout_pool,
            tc.tile_pool(name="psmm", bufs=1, space="PSUM") as ps_mm,
            tc.tile_pool(name="pssc", bufs=2, space="PSUM") as ps_sc,
            tc.tile_pool(name="pssm", bufs=2, space="PSUM") as ps_small,
        ):
            ones_row = const_pool.tile([1, T], F16, tag="ones")
            nc.gpsimd.memset(ones_row[:], 1.0)
            nbias4 = const_pool.tile([128, 1], F32, tag="nbias4")
            nc.gpsimd.memset(nbias4[:], -4.0)

            bias = const_pool.tile([1, 8 * D], F16, tag="bias")
            nc.sync.dma_start(bias[:], bias_all[:])
            BOFF = {("q", "r"): 0, ("q", "i"): 1, ("k", "r"): 2, ("k", "i"): 3,
                    ("v", "r"): 4, ("v", "i"): 5, ("o", "r"): 6, ("o", "i"): 7}

            def brow(p, c):
                o = BOFF[p, c] * D
                return bias[:, o:o + D]

            def load_w(p, eng):
                t = {}
                cs = ("p", "m", "n") if p in ("q", "k") else ("r", "i", "in")
                for ci, c in enumerate(cs):
                    tl = w_pool.tile([128, KT, D], F16, name=f"w{c}",
                                     tag=f"w{ci}")
                    eng.dma_start(
                        tl[:], w[p, c][:].rearrange("(k p) n -> p k n", p=128))
                    t[c] = tl
                return t

            def load_x(p, eng):
                t = {}
                for c in ("r", "i"):
                    tl = x_pool.tile([128, KT, T], F16, name=f"x{c}",
                                     tag=f"x{c}")
                    eng.dma_start(
                        tl[:],
                        xT[f"{p}_{c}"][:].rearrange("(k p) n -> p k n", p=128))
                    t[c] = tl
                return t

            # o-proj weights loaded once up front (const pool)
            wo = {}
            for c in ("r", "i", "in"):
                wo[c] = const_pool.tile([128, KT, D], F16, name=f"wo{c}",
                                        tag=f"wo{c}")
                nc.sync.dma_start(
                    wo[c][:], w["o", c][:].rearrange("(k p) n -> p k n", p=128))

            # v in transposed layout, (vr, vi, 1) interleaved per token,
            # one feature ktile (= head pair) per kp slot
            vT3 = const_pool.tile([128, KT, T, 3], F16, tag="vT3")
            nc.gpsimd.memset(vT3[:, :, :, 2], 1.0)

            # attention output V.T (h-major d' rows), o-proj rhs
            VT = {c: const_pool.tile([128, KT, T], F16, name=f"VT{c}",
                                     tag=f"VT{c}") for c in ("r", "i")}
            # final transposed outputs staged in sbuf
            outS = {c: const_pool.tile([128, KT, T], F16, name=f"outS{c}",
                                       tag=f"outS{c}") for c in ("r", "i")}

            # staging buffers: hst [18, (j, h2, s)] (zero off-block),
            # gst [18, (j, l)] dense; rows = (h2, k)
            hsts = [const_pool.tile([2 * R, CH, 2, E], F16, name=f"hst{i}",
                                    tag=f"hst{i}") for i in range(2)]
            gsts = [const_pool.tile([2 * R, CH, E], F16, name=f"gst{i}",
                                    tag=f"gst{i}") for i in range(2)]
            for i in range(2):
                # zero whole tiles once; staging DMAs only rewrite the data
                # blocks so the off-block zeros persist across chunks
                nc.gpsimd.memset(hsts[i][:], 0.0)

            # per-side per-tt factor tensors g_all/h_all:
            # free layout (hp, h2, k, l) so staging rows (h2,k) merge to one
            # uniform-stride DMA dim on the gst side
            def chain_tiles(side):
                # tag by side only: bufs=2 rotates across token tiles
                s = side[0]
                return gh_pool.tile([128, NHP, 2, R, E], F16,
                                    name=f"gh_{side}", tag=f"gh_{s}")

            # ---------------- phase pieces (emitted interleaved) ----------
            def proj_qk(p, xt, wt, tt):
                """natural-layout projection in sum/diff form for tile tt ->
                psum [128 tok, 0:512]=s1=yr+yi, [512:1024]=s2=yr-yi"""
                ts = bass.ts(tt, PT)
                y = ps_mm.tile([128, 2 * D], F32, name="yqk", tag="mmqk")
                s1 = y[:, 0:D]
                s2 = y[:, D:2 * D]
                for k in range(KT):
                    nc.tensor.matmul(s1, xt["r"][:, k, ts], wt["p"][:, k, :],
                                     start=(k == 0), stop=False)
                for k in range(KT):
                    nc.tensor.matmul(s1, xt["i"][:, k, ts], wt["m"][:, k, :],
                                     start=False, stop=False)
                nc.tensor.matmul(s1, ones_row[:, 0:PT], brow(p, "r"),
                                 start=False, stop=True)
                for k in range(KT):
                    nc.tensor.matmul(s2, xt["r"][:, k, ts], wt["m"][:, k, :],
                                     start=(k == 0), stop=False)
                for k in range(KT):
                    nc.tensor.matmul(s2, xt["i"][:, k, ts], wt["n"][:, k, :],
                                     start=False, stop=False)
                nc.tensor.matmul(s2, ones_row[:, 0:PT], brow(p, "i"),
                                 start=False, stop=True)
                return s1, s2

            def prep_side(side, ps1, ps2, gh, tt):
                """per-token factors: g_k = c_k*a*u^k (side q, with ratios)
                or h_k = b*v^k (side k, pure chain). ps1/ps2 = psum views of
                s1 = yr+yi, s2 = yr-yi; Gp = s1^2, Gm = s2^2 (gpsimd);
                a2' = Gp+Gm = 2m, d = Gp-Gm = 4*yr*yi; u = d*recip(a2') =
                2*yr*yi/m."""
                s1 = prep_pool.tile([128, D], F16, tag="s1")
                s2 = prep_pool.tile([128, D], F16, tag="s2")
                gp = prep_pool.tile([128, D], F16, tag="gp")
                gm = prep_pool.tile([128, D], F16, tag="gm")
                a2 = prep_pool.tile([128, D], F32, tag="a2")
                dd = prep_pool.tile([128, D], F16, tag="dd")
                rr = prep_pool.tile([128, D], F32, tag="rr")
                uu = prep_pool.tile([128, D], F16, tag="uu")
                nc.vector.tensor_copy(s1[:], ps1)
                nc.vector.tensor_copy(s2[:], ps2)
                nc.gpsimd.tensor_mul(gp[:], s1[:], s1[:])
                nc.gpsimd.tensor_mul(gm[:], s2[:], s2[:])
                nc.gpsimd.tensor_add(a2[:], gp[:], gm[:])
                nc.gpsimd.tensor_sub(dd[:], gp[:], gm[:])
                nc.vector.reciprocal(rr[:], a2[:])
                nc.vector.tensor_mul(uu[:], dd[:], rr[:])
                # k0 term: Ln/Exp (single act table):
                # c0*a = exp(0.5*ln(c0^2/2 * a2'))
                lg = prep_pool.tile([128, D], F32, tag="lg")
                c0sq = COEF[0] * COEF[0] / 2 if side == "q" else 0.5
                nc.scalar.activation(lg[:], a2[:], AF.Ln, scale=c0sq)
                g4 = gh[:].rearrange("p hp h2 k l -> p k hp h2 l")
                u4 = uu[:].rearrange("p (hp h2 l) -> p hp h2 l", h2=2, l=E)
                l4 = lg[:].rearrange("p (hp h2 l) -> p hp h2 l", h2=2, l=E)
                nc.scalar.activation(g4[:, 0], l4, AF.Exp, scale=0.5)
                for k in range(1, R):
                    if side == "q":
                        nc.vector.scalar_tensor_tensor(
                            g4[:, k], g4[:, k - 1], RATIO[k - 1], u4,
                            ALU.mult, ALU.mult)
                    else:
                        nc.vector.tensor_mul(g4[:, k], g4[:, k - 1], u4)
                # bounce to DRAM so staging can re-partition by rank
                nc.sync.dma_start(
                    scr[side][tt], gh[:].rearrange("p hp h2 k l -> p (hp h2 k l)"))

            def proj_v(xt, wt, tt, kp):
                """transposed v projection: psum [128 d', 0:128]=yrT,
                [128:256]=yiT for token block tt, feature ktile kp"""
                ts = bass.ts(tt, PT)
                ks = bass.ts(kp, 128)
                y = ps_small.tile([128, 512], F32, name="yv", tag="small")
                yr = y[:, 0:PT]
                yi = y[:, PT:2 * PT]
                for k in range(KT):
                    nc.tensor.matmul(yr, wt["r"][:, k, ks], xt["r"][:, k, ts],
                                     start=(k == 0), stop=False)
                for k in range(KT):
                    nc.tensor.matmul(yr, wt["in"][:, k, ks], xt["i"][:, k, ts],
                                     start=False, stop=False)
                nc.tensor.matmul(yr, brow("v", "r")[:, ks],
                                 ones_row[:, 0:PT], start=False, stop=True)
                for k in range(KT):
                    nc.tensor.matmul(yi, wt["r"][:, k, ks], xt["i"][:, k, ts],
                                     start=(k == 0), stop=False)
                for k in range(KT):
                    nc.tensor.matmul(yi, wt["i"][:, k, ks], xt["r"][:, k, ts],
                                     start=False, stop=False)
                nc.tensor.matmul(yi, brow("v", "i")[:, ks],
                                 ones_row[:, 0:PT], start=False, stop=True)
                # evac into interleaved (vr, vi, 1) columns
                tsl = slice(tt * PT, (tt + 1) * PT)
                nc.vector.tensor_copy(vT3[:, kp, tsl, 0], yr)
                nc.vector.tensor_copy(vT3[:, kp, tsl, 1], yi)

            def stage(tt, hp, ch):
                """stage chunk ch (CH tokens) of (tt, hp) from DRAM scratch:
                1 gst DMA + 2 hst DMAs (per h2); returns (hst, gst) tiles"""
                hst = hsts[(tt * NHP * NCH + hp * NCH + ch) % 2]
                gst = gsts[(tt * NHP * NCH + hp * NCH + ch) % 2]
                jsl = slice(ch * CH, (ch + 1) * CH)
                srcg = scr["q"][tt].rearrange(
                    "j (hp h2 k l) -> hp (h2 k) j l",
                    h2=2, k=R, l=E)[hp][:, jsl, :]
                nc.sync.dma_start(gst[:], srcg)
                srch = scr["k"][tt].rearrange(
                    "j (hp h2 k l) -> hp h2 k j l", h2=2, k=R, l=E)
                for h2 in range(2):
                    nc.sync.dma_start(hst[h2 * R:(h2 + 1) * R, :, h2, :],
                                      srch[hp, h2][:, jsl, :])
                return hst, gst

            def att_chunk(tt, hp, ch, ctr):
                """scores+exp+contraction for CH tokens of (tt, hp)"""
                hst, gst = stage_bufs[tt, hp, ch]
                for r in range(CH // RND):
                    sc = ps_sc.tile([128, RND * E], F32, name="sc", tag="sc")
                    for jj in range(RND):
                        j = r * RND + jj
                        nc.tensor.matmul(
                            sc[:, bass.ts(jj, E)],
                            hst[:, j, :, :].rearrange("r h2 s -> r (h2 s)"),
                            gst[:, j, :])
                    et = et_pool.tile([128, RND * E], F16, name="et", tag="et")
                    # global -4 shift keeps exp in fp16 range (max score ~12);
                    # a constant shift cancels exactly in the softmax ratio
                    nc.scalar.activation(et[:], sc[:], AF.Exp, bias=nbias4[:])
                    for jj in range(RND):
                        j = ch * CH + r * RND + jj
                        jg = tt * PT + j
                        e2 = et[:, bass.ts(jj, E)]
                        nc.tensor.matmul(ctr[0:64, bass.ts(j, 3)],
                                         e2[0:64, :], vT3[0:64, hp, jg, :])
                        nc.tensor.matmul(ctr[64:128, bass.ts(j, 3)],
                                         e2[64:128, :], vT3[64:128, hp, jg, :])

            def att_norm(tt, hp, ctr):
                """normalize ctr -> VT columns for (tt, hp)"""
                c3 = ctr[:].rearrange("p (j c) -> p j c", c=3)
                rcp = prep_pool.tile([128, PT], F32, tag="rcp")
                nc.vector.reciprocal(rcp[:], c3[:, :, 2])
                tsl = slice(tt * PT, (tt + 1) * PT)
                nc.vector.tensor_mul(VT["r"][:, hp, tsl], c3[:, :, 0], rcp[:])
                nc.vector.tensor_mul(VT["i"][:, hp, tsl], c3[:, :, 1], rcp[:])

            def oproj(tt, ko, c):
                """transposed o-projection, out ktile ko, tokens tt"""
                ts = bass.ts(tt, PT)
                ks = bass.ts(ko, 128)
                po = ps_small.tile([128, 512], F32, name="po", tag="small")
                y = po[:, 0:PT]
                wa, wb = ("r", "in") if c == "r" else ("r", "i")
                ra, rb = ("r", "i") if c == "r" else ("i", "r")
                for k in range(KT):
                    nc.tensor.matmul(y, wo[wa][:, k, ks], VT[ra][:, k, ts],
                                     start=(k == 0), stop=False)
                for k in range(KT):
                    nc.tensor.matmul(y, wo[wb][:, k, ks], VT[rb][:, k, ts],
                                     start=False, stop=False)
                nc.tensor.matmul(y, brow("o", c)[:, ks], ones_row[:, 0:PT],
                                 start=False, stop=True)
                nc.vector.tensor_copy(outS[c][:, ko, ts], y)

            # ---------------- emission (interleaved program order) --------
            xq = load_x("q", nc.sync)
            wq = load_w("q", nc.sync)
            xk = load_x("k", nc.gpsimd)
            wk = load_w("k", nc.gpsimd)

            gq = [None] * NTT
            gk = [None] * NTT
            stage_bufs = {}

            # startup: project q/k/v for tt=0/1 and run prep so attention
            # has work ready; then steady-state interleave
            xv = wv = None

            def do_proj_tt(tt):
                nonlocal xv, wv
                s1, s2 = proj_qk("q", xq, wq, tt)
                gq[tt] = chain_tiles(f"q{tt}")
                prep_side("q", s1, s2, gq[tt], tt)
                s1, s2 = proj_qk("k", xk, wk, tt)
                gk[tt] = chain_tiles(f"k{tt}")
                prep_side("k", s1, s2, gk[tt], tt)
                if tt == 0:
                    xv = load_x("v", nc.gpsimd)
                    wv = load_w("v", nc.gpsimd)
                for kp in range(KT):
                    proj_v(xv, wv, tt, kp)

            do_proj_tt(0)
            do_proj_tt(1)

            for tt in range(NTT):
                for hp in range(NHP):
                    for ch in range(NCH):
                        stage_bufs[tt, hp, ch] = stage(tt, hp, ch)
                    ctr = ps_small.tile([128, PT * 3], F32, name="ctr",
                                        tag="small")
                    for ch in range(NCH):
                        att_chunk(tt, hp, ch, ctr)
                    att_norm(tt, hp, ctr)
                    # interleave next-tile projections between head pairs
                    if tt + 2 < NTT and hp == 0:
                        do_proj_tt(tt + 2)
                # o-projection for finished token tile
                for ko in range(KT):
                    oproj(tt, ko, "r")
                    oproj(tt, ko, "i")

            nc.sync.dma_start(
                outT_r[:].rearrange("(k p) n -> p k n", p=128), outS["r"][:])
            nc.sync.dma_start(
                outT_i[:].rearrange("(k p) n -> p k n", p=128), outS["i"][:])

    nc.compile()
    return nc


_NC_CACHE = None


def _get_module():
    global _NC_CACHE
    if _NC_CACHE is None:
        _patch_act_tables()
        _NC_CACHE = _build_module()
    return _NC_CACHE


def _prep_inputs(inputs):
    """host-side shard/layout prep -> list of 8 per-core input maps"""
    TT = B * L
    f16 = np.float16
    xs = {nm: np.ascontiguousarray(
        np.asarray(inputs[nm]).reshape(TT, D).T.astype(f16))
        for nm in ("q_r", "q_i", "k_r", "k_i", "v_r", "v_i")}
    # h-major feature permutation d' = h*64 + l  <->  d = l*H + h
    perm = np.empty(D, np.int64)
    for h in range(H):
        for l in range(E):
            perm[h * E + l] = l * H + h
    common = {}
    brows = np.empty((8, D), np.float32)
    BOFF = {"q": 0, "k": 2, "v": 4, "o": 6}
    for p in ("q", "k", "v", "o"):
        wr = np.asarray(inputs[f"w{p}_r"]).astype(np.float32)
        wi = np.asarray(inputs[f"w{p}_i"]).astype(np.float32)
        br = np.asarray(inputs[f"b{p}_r"]).astype(np.float32)
        bi = np.asarray(inputs[f"b{p}_i"]).astype(np.float32)
        wrT = wr.T
        wiT = wi.T
        if p in ("q", "k"):
            # sum/diff form: s1 = xr@W+ + xi@W- + 2br,
            #                s2 = xr@W- - xi@W+ - 2bi
            wp = (wrT + wiT)[:, perm]
            wm = (wrT - wiT)[:, perm]
            common[f"w_{p}_p"] = np.ascontiguousarray(wp.astype(f16))
            common[f"w_{p}_m"] = np.ascontiguousarray(wm.astype(f16))
            common[f"w_{p}_n"] = np.ascontiguousarray((-wp).astype(f16))
            brows[BOFF[p]] = (2 * br)[perm]
            brows[BOFF[p] + 1] = (-2 * bi)[perm]
        else:
            if p == "o":
                wrT = wrT[perm, :]     # permuted contraction rows
                wiT = wiT[perm, :]
                b_r, b_i = br - bi, br + bi
            else:
                wrT = wrT[:, perm]     # permuted output features
                wiT = wiT[:, perm]
                b_r, b_i = (br - bi)[perm], (br + bi)[perm]
            common[f"w_{p}_r"] = np.ascontiguousarray(wrT.astype(f16))
            common[f"w_{p}_i"] = np.ascontiguousarray(wiT.astype(f16))
            common[f"w_{p}_in"] = np.ascontiguousarray((-wiT).astype(f16))
            brows[BOFF[p]] = b_r
            brows[BOFF[p] + 1] = b_i
    common["bias_all"] = brows.reshape(1, 8 * D).astype(f16)
    maps = []
    for c in range(NCORES):
        m = dict(common)
        sl = slice(c * T, (c + 1) * T)
        for nm, arr in xs.items():
            m[f"x_{nm}_T"] = np.ascontiguousarray(arr[:, sl])
        maps.append(m)
    return maps


def kernel(**inputs):
    nc = _get_module()
    maps = _prep_inputs(inputs)
    res = run_bass_kernel_spmd(nc, maps, core_ids=list(range(NCORES)))
    out_r = np.concatenate(
        [np.asarray(res.results[c]["outT_r"], dtype=np.float32).T
         for c in range(NCORES)], axis=0).reshape(B, L, D)
    out_i = np.concatenate(
        [np.asarray(res.results[c]["outT_i"], dtype=np.float32).T
         for c in range(NCORES)], axis=0).reshape(B, L, D)
    return out_r, out_i


# revision 20
# speedup vs baseline: 2.3683x; 1.0855x over previous
"""ComplexAttentionLayer Trainium2 kernel, v3 (8-core data-parallel).

Math (per token t, head h; E=64): with q = qr+i*qi, k = kr+i*ki per (h,l):
  m = qr^2+qi^2, p = 2*qr*qi, n = kr^2+ki^2, t2 = 2*kr*ki
  |s[l,s]|^2 = m[l]*n[s] - p[l]*t2[s] = (a_l b_s)^2 (1 - u_l v_s),
  a = sqrt(m), b = sqrt(n), u = p/m, v = t2/n  (|u|,|v| <= 1)
  attn = softmax_s exp(|s|), out = attn @ v.

Key trick: |s| = a_l b_s sqrt(1 - u_l v_s) is approximated by a degree-8
polynomial sqrt(1-z) ~= sum_k c_k z^k (max end-to-end rel err ~2.4e-3 in
fp16), making |s| itself a rank-9 PE matmul over per-token factor vectors
  g_k[l] = c_k a u^k (q side),  h_k[s] = b v^k (k side).
The scalar engine then does a SINGLE exp pass over the 16.8M scores
(vs ln/exp/exp in v2), and no per-element sqrt exists anywhere.

Layouts: q/k projections natural [token-part, feat]; v and o projections
transposed [feat-part, token] (weights permuted host-side to h-major
d' = h*64 + e). Per (token-tile, head-pair hp) the score matmul is
  out[(h2,s), l] = hst[K=18, (h2,s)].T @ gst[K=18, (j,l)]
with hst rows (h2,k) = h_k of head 2hp+h2 (zero off-block) staged by 3-dim
batched SBUF->SBUF DMAs from chain-multiply outputs, and the contraction
  ctr[l, (j,3)] += Et[(h2,s), j-slice].T @ vT3[(h2,s), j*3:(j+1)*3]
uses v in transposed layout with (vr, vi, 1) interleaved columns; the ones
column yields the softmax denominators.
"""

import numpy as np

import concourse.bass as bass
import concourse.tile as tile
from concourse import bacc, mybir
from concourse.bass_utils import run_bass_kernel_spmd

AF = mybir.ActivationFunctionType
ALU = mybir.AluOpType
F32 = mybir.dt.float32
F16 = mybir.dt.float16

B, L, D, H = 4, 1024, 512, 8
E = D // H           # 64
NCORES = 8
T = B * L // NCORES  # 512 tokens per core
PT = 128             # tokens per tile
NTT = T // PT        # 4 token tiles
KT = D // 128        # 4 k-tiles per weight
NHP = H // 2         # 4 head pairs
R = 9                # score rank (poly degree 8)
CH = 64              # tokens per staging chunk
NCH = PT // CH       # 2 chunks per token tile
RND = 16             # tokens per psum round

# degree-8 lstsq fit of sqrt(1-z) on [-1,1] (see module docstring)
COEF = [0.9990085414, -0.4886256302, -0.0785392818, -0.1987831962,
        -0.3640738545, 0.3670958221, 0.6909310557, -0.3454001713,
        -0.5032298768]
RATIO = [COEF[k + 1] / COEF[k] for k in range(R - 1)]


def _patch_act_tables():
    """Keep Ln/Exp/Copy/Identity only in natural_log_exp_and_others so the
    table-load pass picks the one set covering all our activations (this
    kernel only uses Ln and Exp, which genuinely share that table)."""
    from concourse import hw_specs
    orig = hw_specs.get_activation_tables
    ours = {AF.Ln, AF.Exp, AF.Copy, AF.Identity}

    def patched(module_arch):
        tabs = orig(module_arch)
        if "natural_log_exp_and_others" in tabs:
            for name, fns in tabs.items():
                if name != "natural_log_exp_and_others":
                    tabs[name] = fns - ours
        return tabs

    bacc.get_activation_tables = patched


def _build_module():
    nc = bacc.Bacc()

    xT = {}
    for nm in ("q_r", "q_i", "k_r", "k_i", "v_r", "v_i"):
        xT[nm] = nc.declare_dram_parameter(f"x_{nm}_T", [D, T], F16,
                                           isOutput=False)
    w = {}
    for p in ("q", "k"):
        # p = (wr+wi).T, m = (wr-wi).T, n = -(wr+wi).T  (sum/diff form so the
        # projection outputs s1 = yr+yi, s2 = yr-yi directly)
        for c in ("p", "m", "n"):
            w[p, c] = nc.declare_dram_parameter(f"w_{p}_{c}", [D, D], F16,
                                                isOutput=False)
    for p in ("v", "o"):
        for c in ("r", "i", "in"):  # r = w_r.T, i = w_i.T, in = -w_i.T
            w[p, c] = nc.declare_dram_parameter(f"w_{p}_{c}", [D, D], F16,
                                                isOutput=False)
    bias_all = nc.declare_dram_parameter("bias_all", [1, 8 * D], F16,
                                         isOutput=False)
    # DRAM bounce scratch for the partition->free staging transpose
    # (SBUF->SBUF DMAs cannot move data across partitions multi-dim)
    scr = {s: nc.declare_dram_parameter(f"scr_{s}", [NTT, 128, NHP * 2 * R * E],
                                        F16, isOutput=True)
           for s in ("q", "k")}
    outT_r = nc.declare_dram_parameter("outT_r", [D, T], F16, isOutput=True)
    outT_i = nc.declare_dram_parameter("outT_i", [D, T], F16, isOutput=True)

    with tile.TileContext(nc) as tc:
        with (
            tc.tile_pool(name="const", bufs=1) as const_pool,
            tc.tile_pool(name="xin", bufs=2) as x_pool,
            tc.tile_pool(name="wgt", bufs=2) as w_pool,
            tc.tile_pool(name="prep", bufs=2) as prep_pool,
            tc.tile_pool(name="gh", bufs=2) as gh_pool,
            tc.tile_pool(name="stage", bufs=2) as stage_pool,
            tc.tile_pool(name="et", bufs=3) as et_pool,
            tc.tile_pool(name="vout", bufs=1) as vout_pool,
            tc.tile_pool(name="psmm", bufs=1, space="PSUM") as ps_mm,
            tc.tile_pool(name="pssc", bufs=2, space="PSUM") as ps_sc,
            tc.tile_pool(name="pssm", bufs=2, space="PSUM") as ps_small,
        ):
            ones_row = const_pool.tile([1, T], F16, tag="ones")
            nc.gpsimd.memset(ones_row[:], 1.0)
            nbias4 = const_pool.tile([128, 1], F32, tag="nbias4")
            nc.gpsimd.memset(nbias4[:], -4.0)

            bias = const_pool.tile([1, 8 * D], F16, tag="bias")
            nc.sync.dma_start(bias[:], bias_all[:])
            BOFF = {("q", "r"): 0, ("q", "i"): 1, ("k", "r"): 2, ("k", "i"): 3,
                    ("v", "r"): 4, ("v", "i"): 5, ("o", "r"): 6, ("o", "i"): 7}

            def brow(p, c):
                o = BOFF[p, c] * D
                return bias[:, o:o + D]

            def load_w(p, eng):
                t = {}
                cs = ("p", "m", "n") if p in ("q", "k") else ("r", "i", "in")
                for ci, c in enumerate(cs):
                    tl = w_pool.tile([128, KT, D], F16, name=f"w{c}",
                                     tag=f"w{ci}")
                    eng.dma_start(
                        tl[:], w[p, c][:].rearrange("(k p) n -> p k n", p=128))
                    t[c] = tl
                return t

            def load_x(p, eng):
                t = {}
                for c in ("r", "i"):
                    tl = x_pool.tile([128, KT, T], F16, name=f"x{c}",
                                     tag=f"x{c}")
                    eng.dma_start(
                        tl[:],
                        xT[f"{p}_{c}"][:].rearrange("(k p) n -> p k n", p=128))
                    t[c] = tl
                return t

            # o-proj weights loaded once up front (const pool)
            wo = {}
            for c in ("r", "i", "in"):
                wo[c] = const_pool.tile([128, KT, D], F16, name=f"wo{c}",
                                        tag=f"wo{c}")
                nc.sync.dma_start(
                    wo[c][:], w["o", c][:].rearrange("(k p) n -> p k n", p=128))

            # v in transposed layout, (vr, vi, 1) interleaved per token,
            # one feature ktile (= head pair) per kp slot
            vT3 = const_pool.tile([128, KT, T, 3], F16, tag="vT3")
            nc.gpsimd.memset(vT3[:, :, :, 2], 1.0)

            # attention output V.T (h-major d' rows), o-proj rhs
            VT = {c: const_pool.tile([128, KT, T], F16, name=f"VT{c}",
                                     tag=f"VT{c}") for c in ("r", "i")}
            # final transposed outputs staged in sbuf
            outS = {c: const_pool.tile([128, KT, T], F16, name=f"outS{c}",
                                       tag=f"outS{c}") for c in ("r", "i")}

            # staging buffers: hst [18, (j, h2, s)] (zero off-block),
            # gst [18, (j, l)] dense; rows = (h2, k)
            hsts = [const_pool.tile([2 * R, CH, 2, E], F16, name=f"hst{i}",
                                    tag=f"hst{i}") for i in range(2)]
            gsts = [const_pool.tile([2 * R, CH, E], F16, name=f"gst{i}",
                                    tag=f"gst{i}") for i in range(2)]
            for i in range(2):
                # zero whole tiles once; staging DMAs only rewrite the data
                # blocks so the off-block zeros persist across chunks
                nc.scalar.memzero(hsts[i][:])

            # per-side per-tt factor tensors g_all/h_all:
            # free layout (hp, h2, k, l) so staging rows (h2,k) merge to one
            # uniform-stride DMA dim on the gst side
            def chain_tiles(side):
                # tag by side only: bufs=2 rotates across token tiles
                s = side[0]
                return gh_pool.tile([128, NHP, 2, R, E], F16,
                                    name=f"gh_{side}", tag=f"gh_{s}")

            # ---------------- phase pieces (emitted interleaved) ----------
            def proj_qk(p, xt, wt, tt):
                """natural-layout projection in sum/diff form for tile tt ->
                psum [128 tok, 0:512]=s1=yr+yi, [512:1024]=s2=yr-yi"""
                ts = bass.ts(tt, PT)
                y = ps_mm.tile([128, 2 * D], F32, name="yqk", tag="mmqk")
                s1 = y[:, 0:D]
                s2 = y[:, D:2 * D]
                for k in range(KT):
                    nc.tensor.matmul(s1, xt["r"][:, k, ts], wt["p"][:, k, :],
                                     start=(k == 0), stop=False)
                for k in range(KT):
                    nc.tensor.matmul(s1, xt["i"][:, k, ts], wt["m"][:, k, :],
                                     start=False, stop=False)
                nc.tensor.matmul(s1, ones_row[:, 0:PT], brow(p, "r"),
                                 start=False, stop=True)
                for k in range(KT):
                    nc.tensor.matmul(s2, xt["r"][:, k, ts], wt["m"][:, k, :],
                                     start=(k == 0), stop=False)
                for k in range(KT):
                    nc.tensor.matmul(s2, xt["i"][:, k, ts], wt["n"][:, k, :],
                                     start=False, stop=False)
                nc.tensor.matmul(s2, ones_row[:, 0:PT], brow(p, "i"),
                                 start=False, stop=True)
                return s1, s2

            def prep_side(side, ps1, ps2, gh, tt):
                """per-token factors: g_k = c_k*a*u^k (side q, with ratios)
                or h_k = b*v^k (side k, pure chain). ps1/ps2 = psum views of
                s1 = yr+yi, s2 = yr-yi; Gp = s1^2, Gm = s2^2 (gpsimd);
                a2' = Gp+Gm = 2m, d = Gp-Gm = 4*yr*yi; u = d*recip(a2') =
                2*yr*yi/m."""
                s1 = prep_pool.tile([128, D], F16, tag="s1")
                s2 = prep_pool.tile([128, D], F16, tag="s2")
                gp = prep_pool.tile([128, D], F16, tag="gp")
                gm = prep_pool.tile([128, D], F16, tag="gm")
                a2 = prep_pool.tile([128, D], F32, tag="a2")
                dd = prep_pool.tile([128, D], F16, tag="dd")
                rr = prep_pool.tile([128, D], F32, tag="rr")
                uu = prep_pool.tile([128, D], F16, tag="uu")
                nc.scalar.copy(s1[:], ps1)
                nc.scalar.copy(s2[:], ps2)
                nc.gpsimd.tensor_mul(gp[:], s1[:], s1[:])
                nc.gpsimd.tensor_mul(gm[:], s2[:], s2[:])
                nc.gpsimd.tensor_add(a2[:], gp[:], gm[:])
                nc.gpsimd.tensor_sub(dd[:], gp[:], gm[:])
                nc.vector.reciprocal(rr[:], a2[:])
                nc.vector.tensor_mul(uu[:], dd[:], rr[:])
                # k0 term: Ln/Exp (single act table):
                # c0*a = exp(0.5*ln(c0^2/2 * a2'))
                lg = prep_pool.tile([128, D], F16, tag="lg")
                c0sq = COEF[0] * COEF[0] / 2 if side == "q" else 0.5
                nc.scalar.activation(lg[:], a2[:], AF.Ln, scale=c0sq)
                g4 = gh[:].rearrange("p hp h2 k l -> p k hp h2 l")
                u4 = uu[:].rearrange("p (hp h2 l) -> p hp h2 l", h2=2, l=E)
                l4 = lg[:].rearrange("p (hp h2 l) -> p hp h2 l", h2=2, l=E)
                nc.scalar.activation(g4[:, 0], l4, AF.Exp, scale=0.5)
                for k in range(1, R):
                    if side == "q":
                        nc.vector.scalar_tensor_tensor(
                            g4[:, k], g4[:, k - 1], RATIO[k - 1], u4,
                            ALU.mult, ALU.mult)
                    else:
                        nc.vector.tensor_mul(g4[:, k], g4[:, k - 1], u4)
                # bounce to DRAM so staging can re-partition by rank
                nc.sync.dma_start(
                    scr[side][tt], gh[:].rearrange("p hp h2 k l -> p (hp h2 k l)"))

            def proj_v(xt, wt, tt, kp):
                """transposed v projection: psum [128 d', 0:128]=yrT,
                [128:256]=yiT for token block tt, feature ktile kp"""
                ts = bass.ts(tt, PT)
                ks = bass.ts(kp, 128)
                y = ps_small.tile([128, 512], F32, name="yv", tag="small")
                yr = y[:, 0:PT]
                yi = y[:, PT:2 * PT]
                for k in range(KT):
                    nc.tensor.matmul(yr, wt["r"][:, k, ks], xt["r"][:, k, ts],
                                     start=(k == 0), stop=False)
                for k in range(KT):
                    nc.tensor.matmul(yr, wt["in"][:, k, ks], xt["i"][:, k, ts],
                                     start=False, stop=False)
                nc.tensor.matmul(yr, brow("v", "r")[:, ks],
                                 ones_row[:, 0:PT], start=False, stop=True)
                for k in range(KT):
                    nc.tensor.matmul(yi, wt["r"][:, k, ks], xt["i"][:, k, ts],
                                     start=(k == 0), stop=False)
                for k in range(KT):
                    nc.tensor.matmul(yi, wt["i"][:, k, ks], xt["r"][:, k, ts],
                                     start=False, stop=False)
                nc.tensor.matmul(yi, brow("v", "i")[:, ks],
                                 ones_row[:, 0:PT], start=False, stop=True)
                # evac into interleaved (vr, vi, 1) columns
                tsl = slice(tt * PT, (tt + 1) * PT)
                nc.scalar.copy(vT3[:, kp, tsl, 0], yr)
                nc.scalar.copy(vT3[:, kp, tsl, 1], yi)

            def stage(tt, hp, ch):
                """stage chunk ch (CH tokens) of (tt, hp) from DRAM scratch:
                1 gst DMA + 2 hst DMAs (per h2); returns (hst, gst) tiles"""
                hst = hsts[(tt * NHP * NCH + hp * NCH + ch) % 2]
                gst = gsts[(tt * NHP * NCH + hp * NCH + ch) % 2]
                jsl = slice(ch * CH, (ch + 1) * CH)
                srcg = scr["q"][tt].rearrange(
                    "j (hp h2 k l) -> hp (h2 k) j l",
                    h2=2, k=R, l=E)[hp][:, jsl, :]
                nc.sync.dma_start(gst[:], srcg)
                srch = scr["k"][tt].rearrange(
                    "j (hp h2 k l) -> hp h2 k j l", h2=2, k=R, l=E)
                for h2 in range(2):
                    nc.sync.dma_start(hst[h2 * R:(h2 + 1) * R, :, h2, :],
                                      srch[hp, h2][:, jsl, :])
                return hst, gst

            def att_chunk(tt, hp, ch, ctr):
                """scores+exp+contraction for CH tokens of (tt, hp)"""
                hst, gst = stage_bufs[tt, hp, ch]
                for r in range(CH // RND):
                    sc = ps_sc.tile([128, RND * E], F32, name="sc", tag="sc")
                    for jj in range(RND):
                        j = r * RND + jj
                        nc.tensor.matmul(
                            sc[:, bass.ts(jj, E)],
                            hst[:, j, :, :].rearrange("r h2 s -> r (h2 s)"),
                            gst[:, j, :])
                    et = et_pool.tile([128, RND * E], F16, name="et", tag="et")
                    # global -4 shift keeps exp in fp16 range (max score ~12);
                    # a constant shift cancels exactly in the softmax ratio
                    nc.scalar.activation(et[:], sc[:], AF.Exp, bias=nbias4[:])
                    for jj in range(RND):
                        j = ch * CH + r * RND + jj
                        jg = tt * PT + j
                        e2 = et[:, bass.ts(jj, E)]
                        nc.tensor.matmul(ctr[0:64, bass.ts(j, 3)],
                                         e2[0:64, :], vT3[0:64, hp, jg, :])
                        nc.tensor.matmul(ctr[64:128, bass.ts(j, 3)],
                                         e2[64:128, :], vT3[64:128, hp, jg, :])

            def att_norm(tt, hp, ctr):
                """normalize ctr -> VT columns for (tt, hp)"""
                c3 = ctr[:].rearrange("p (j c) -> p j c", c=3)
                rcp = prep_pool.tile([128, PT], F32, tag="rcp")
                nc.vector.reciprocal(rcp[:], c3[:, :, 2])
                tsl = slice(tt * PT, (tt + 1) * PT)
                nc.vector.tensor_mul(VT["r"][:, hp, tsl], c3[:, :, 0], rcp[:])
                nc.vector.tensor_mul(VT["i"][:, hp, tsl], c3[:, :, 1], rcp[:])

            def oproj(tt, ko, c):
                """transposed o-projection, out ktile ko, tokens tt"""
                ts = bass.ts(tt, PT)
                ks = bass.ts(ko, 128)
                po = ps_small.tile([128, 512], F32, name="po", tag="small")
                y = po[:, 0:PT]
                wa, wb = ("r", "in") if c == "r" else ("r", "i")
                ra, rb = ("r", "i") if c == "r" else ("i", "r")
                for k in range(KT):
                    nc.tensor.matmul(y, wo[wa][:, k, ks], VT[ra][:, k, ts],
                                     start=(k == 0), stop=False)
                for k in range(KT):
                    nc.tensor.matmul(y, wo[wb][:, k, ks], VT[rb][:, k, ts],
                                     start=False, stop=False)
                nc.tensor.matmul(y, brow("o", c)[:, ks], ones_row[:, 0:PT],
                                 start=False, stop=True)
                nc.vector.tensor_copy(outS[c][:, ko, ts], y)

            # ---------------- emission (interleaved program order) --------
            xq = load_x("q", nc.sync)
            wq = load_w("q", nc.sync)
            xk = load_x("k", nc.scalar)
            wk = load_w("k", nc.scalar)

            gq = [None] * NTT
            gk = [None] * NTT
            stage_bufs = {}

            # startup: project q/k/v for tt=0/1 and run prep so attention
            # has work ready; then steady-state interleave
            xv = wv = None

            def do_proj_tt(tt):
                nonlocal xv, wv
                s1, s2 = proj_qk("q", xq, wq, tt)
                gq[tt] = chain_tiles(f"q{tt}")
                prep_side("q", s1, s2, gq[tt], tt)
                s1, s2 = proj_qk("k", xk, wk, tt)
                gk[tt] = chain_tiles(f"k{tt}")
                prep_side("k", s1, s2, gk[tt], tt)
                if tt == 0:
                    xv = load_x("v", nc.scalar)
                    wv = load_w("v", nc.scalar)
                for kp in range(KT):
                    proj_v(xv, wv, tt, kp)

            do_proj_tt(0)

            for tt in range(NTT):
                for hp in range(NHP):
                    for ch in range(NCH):
                        stage_bufs[tt, hp, ch] = stage(tt, hp, ch)
                    ctr = ps_small.tile([128, PT * 3], F32, name="ctr",
                                        tag="small")
                    for ch in range(NCH):
                        att_chunk(tt, hp, ch, ctr)
                    att_norm(tt, hp, ctr)
                    # interleave next-tile projections between head pairs
                    if tt + 1 < NTT and hp == 0:
                        do_proj_tt(tt + 1)
                # o-projection for finished token tile
                for ko in range(KT):
                    oproj(tt, ko, "r")
                    oproj(tt, ko, "i")

            nc.sync.dma_start(
                outT_r[:].rearrange("(k p) n -> p k n", p=128), outS["r"][:])
            nc.sync.dma_start(
                outT_i[:].rearrange("(k p) n -> p k n", p=128), outS["i"][:])

    nc.compile()
    return nc


_NC_CACHE = None


def _get_module():
    global _NC_CACHE
    if _NC_CACHE is None:
        _patch_act_tables()
        _NC_CACHE = _build_module()
    return _NC_CACHE


def _prep_inputs(inputs):
    """host-side shard/layout prep -> list of 8 per-core input maps"""
    TT = B * L
    f16 = np.float16
    xs = {nm: np.ascontiguousarray(
        np.asarray(inputs[nm]).reshape(TT, D).T.astype(f16))
        for nm in ("q_r", "q_i", "k_r", "k_i", "v_r", "v_i")}
    # h-major feature permutation d' = h*64 + l  <->  d = l*H + h
    perm = np.empty(D, np.int64)
    for h in range(H):
        for l in range(E):
            perm[h * E + l] = l * H + h
    common = {}
    brows = np.empty((8, D), np.float32)
    BOFF = {"q": 0, "k": 2, "v": 4, "o": 6}
    for p in ("q", "k", "v", "o"):
        wr = np.asarray(inputs[f"w{p}_r"]).astype(np.float32)
        wi = np.asarray(inputs[f"w{p}_i"]).astype(np.float32)
        br = np.asarray(inputs[f"b{p}_r"]).astype(np.float32)
        bi = np.asarray(inputs[f"b{p}_i"]).astype(np.float32)
        wrT = wr.T
        wiT = wi.T
        if p in ("q", "k"):
            # sum/diff form: s1 = xr@W+ + xi@W- + 2br,
            #                s2 = xr@W- - xi@W+ - 2bi
            wp = (wrT + wiT)[:, perm]
            wm = (wrT - wiT)[:, perm]
            common[f"w_{p}_p"] = np.ascontiguousarray(wp.astype(f16))
            common[f"w_{p}_m"] = np.ascontiguousarray(wm.astype(f16))
            common[f"w_{p}_n"] = np.ascontiguousarray((-wp).astype(f16))
            brows[BOFF[p]] = (2 * br)[perm]
            brows[BOFF[p] + 1] = (-2 * bi)[perm]
        else:
            if p == "o":
                wrT = wrT[perm, :]     # permuted contraction rows
                wiT = wiT[perm, :]
                b_r, b_i = br - bi, br + bi
            else:
                wrT = wrT[:, perm]     # permuted output features
                wiT = wiT[:, perm]
                b_r, b_i = (br - bi)[perm], (br + bi)[perm]
            common[f"w_{p}_r"] = np.ascontiguousarray(wrT.astype(f16))
            common[f"w_{p}_i"] = np.ascontiguousarray(wiT.astype(f16))
            common[f"w_{p}_in"] = np.ascontiguousarray((-wiT).astype(f16))
            brows[BOFF[p]] = b_r
            brows[BOFF[p] + 1] = b_i
    common["bias_all"] = brows.reshape(1, 8 * D).astype(f16)
    maps = []
    for c in range(NCORES):
        m = dict(common)
        sl = slice(c * T, (c + 1) * T)
        for nm, arr in xs.items():
            m[f"x_{nm}_T"] = np.ascontiguousarray(arr[:, sl])
        maps.append(m)
    return maps


def kernel(**inputs):
    nc = _get_module()
    maps = _prep_inputs(inputs)
    res = run_bass_kernel_spmd(nc, maps, core_ids=list(range(NCORES)))
    out_r = np.concatenate(
        [np.asarray(res.results[c]["outT_r"], dtype=np.float32).T
         for c in range(NCORES)], axis=0).reshape(B, L, D)
    out_i = np.concatenate(
        [np.asarray(res.results[c]["outT_i"], dtype=np.float32).T
         for c in range(NCORES)], axis=0).reshape(B, L, D)
    return out_r, out_i
